# revision 7
# baseline (speedup 1.0000x reference)
"""Trainium2 Bass kernel for nn_ConvLayer: 3x3 conv (stride 1, pad 1) + per-channel offset.

Problem: x[32,64,56,56] (*) w[128,64,3,3] + offset[128,1,1] -> out[32,128,56,56], fp32.

Strategy (8 NeuronCores, data-parallel over batch, 4 images/core):
  - Conv as 9 shifted matmuls (one per 3x3 tap) accumulated in fp32 PSUM.
  - CIN=64 -> each tap is a contract-64 matmul = half the 128x128 PE array.
    Two images are processed CONCURRENTLY via 64x128 row tiling: image A's
    channels live in SBUF partitions 0-63 (PE tile (0,0)), image B's in
    partitions 64-127 (PE tile (64,0)). Each accumulates into its own PSUM
    bank, reaching full PE-array packing with no data duplication.
  - All device-side tensors are fp16: x and weights are downcast on host
    (error budget: fp16 products accumulated in fp32 PSUM give ~5e-4 rel
    err vs the 2e-2 gate), and the conv output is stored to HBM as fp16 and
    upcast to fp32 on host. This halves HBM traffic (10.5 -> 5.1 MB/core),
    which removed the DMA-bound head/tail stalls of the fp32 version.
  - Host pre-pads x to a 57-stride grid: ONE zero column is shared between
    consecutive rows (right-pad of row r == left-pad of row r+1), plus one
    zero row above and below. Every tap read is then a single contiguous
    shifted window. Weights are pre-transposed to [cin, tap, k] (lhsT
    layout) and duplicated into both partition halves.
  - Output columns are produced on the padded 57-wide grid; the PSUM->SBUF
    eviction (ScalarE for image A, VectorE for image B) compacts to the
    dense 56-wide grid, fuses the per-channel offset add, and downcasts to
    fp16, so the store DMA is small and fully contiguous.
"""

import numpy as np
from contextlib import ExitStack

import concourse.bass as bass
import concourse.tile as tile
from concourse import bacc, mybir
from concourse.bass_utils import run_bass_kernel_spmd

# Problem constants (hardcoded per contract).
B, CIN, HW, K = 32, 64, 56, 128
NCORES = 8
BPC = B // NCORES          # images per core
HP = HW + 1                # padded row stride: 57 (one shared pad col)
BASE = 1                   # element (row, col) lives at BASE + (row+1)*HP + col
NPAD = BASE + (HW + 2) * HP + 4   # 58 padded rows + tap-read slack: 3312
NOUT = HW * HW             # 3136
ROWS_PER_CHUNK = 7
CHUNK = ROWS_PER_CHUNK * HP     # 399 <= 512 (one PSUM bank, fp32)
DCHUNK = ROWS_PER_CHUNK * HW    # 392 dense output cols per chunk
NCHUNKS = HW // ROWS_PER_CHUNK  # 8
TAPS = 9
F16 = mybir.dt.float16
F32 = mybir.dt.float32

_NC_CACHE = None


def _conv_kernel(ctx: ExitStack, tc: "tile.TileContext", out_ap, xp_ap, w2_ap, off_ap):
    nc = tc.nc
    singles = ctx.enter_context(tc.tile_pool(name="singles", bufs=1))
    xpool = ctx.enter_context(tc.tile_pool(name="xpool", bufs=2))
    opool = ctx.enter_context(tc.tile_pool(name="opool", bufs=2))
    psum = ctx.enter_context(tc.tile_pool(name="psum", bufs=8, space="PSUM"))

    # Chunk groups per PSUM allocation: 2 banks per chunk (2 halves). The
    # first and last groups are single chunks: the first so its (small) input
    # slice lands ASAP and the matmul stream starts early, the last so the
    # end-of-kernel eviction+store tail is short.
    groups = [(0,), (1, 2), (3, 4), (5, 6), (7,)]
    # x-load slices (all on the Sync HWDGE ring, in consumption order, one
    # slice per chunk group). Chunk c reads cols < 399*c + 515.
    xbounds = [0, 515, 1314, 2112, 2910, NPAD]

    # Weights as lhsT [c, tap, k], duplicated across both partition halves.
    # One DMA, FIRST in the Sync ring's order: the x slices share that ring
    # and each SDMA engine drains its ring in order, so the weights complete
    # before the x stream — on a separate ring they would crawl at half rate
    # until ~12us (packet round-robin) and stall the first taps.
    w_sb = singles.tile([128, TAPS, K], F16)
    nc.sync.dma_start(w_sb[:], w2_ap[:])
    off_sb = singles.tile([128, 1], F32)
    nc.scalar.dma_start(off_sb[:], off_ap[:])

    # PE warmup: cheap bf16 matmuls on scratch keep TensorE busy through the
    # whole input-DMA head (~4us: body start ~7us to w+x landing ~11us, worse
    # when an SDMA engine wakes late). This both opens the HAM clock gate
    # (1.2 -> 2.4 GHz at ~3.4us of sustained PE activity) and avoids PE-idle
    # gaps that would reset the HAM window, so the real matmul stream starts
    # fully warm. ~426ns each at the cold clock.
    scratch = singles.tile([128, 512], mybir.dt.bfloat16)
    nc.vector.memset(scratch[:], 0.0)
    ps_warm = psum.tile([128, 512], F32, tag="ps", name="ps_warm")
    for _ in range(9):
        nc.tensor.matmul(
            ps_warm[:], lhsT=scratch[0:64, 0:128], rhs=scratch[0:64, :],
            start=True, stop=True,
        )

    for pair in range(BPC // 2):
        b0 = 2 * pair
        # Both images of the pair side by side: [2, CIN, NPAD] -> [128, NPAD],
        # split into column slices so early chunk groups start ASAP.
        x_t = xpool.tile([128, NPAD], F16, tag="x")
        xsrc = xp_ap[b0 : b0 + 2].rearrange("b c n -> (b c) n")
        for s in range(len(xbounds) - 1):
            nc.sync.dma_start(
                x_t[:, xbounds[s] : xbounds[s + 1]],
                xsrc[:, xbounds[s] : xbounds[s + 1]],
            )
        o_sb = [
            opool.tile([128, NOUT], F16, tag="oA", name=f"oA_{pair}"),
            opool.tile([128, NOUT], F16, tag="oB", name=f"oB_{pair}"),
        ]

        for g, grp in enumerate(groups):
            ps = {}
            for half in (0, 1):
                for c in grp:
                    ps[(half, c)] = psum.tile(
                        [128, CHUNK], F32, tag="ps", name=f"ps_{pair}_{half}_{c}"
                    )
            for t in range(TAPS):
                kh, kw = divmod(t, 3)
                o = kh * HP + kw
                st, sp = (t == 0), (t == TAPS - 1)
                for half in (0, 1):
                    lo, hi = 64 * half, 64 * half + 64
                    for c in grp:
                        nc.tensor.matmul(
                            ps[(half, c)][:],
                            lhsT=w_sb[lo:hi, t, :],
                            rhs=x_t[lo:hi, o + CHUNK * c : o + CHUNK * c + CHUNK],
                            start=st,
                            stop=sp,
                        )
            # Evict: compact 57-stride padded rows to 56-wide dense rows, add
            # the per-channel offset, downcast to fp16. Image A on ScalarE,
            # image B on VectorE (they hit different PSUM banks in parallel).
            for c in grp:
                pa = ps[(0, c)].rearrange("p (r x) -> p r x", x=HP)[:, :, 0:HW]
                oa = o_sb[0][:, c * DCHUNK : (c + 1) * DCHUNK].rearrange(
                    "p (r x) -> p r x", x=HW
                )
                nc.scalar.add(oa, pa, off_sb)
                pb = ps[(1, c)].rearrange("p (r x) -> p r x", x=HP)[:, :, 0:HW]
                ob = o_sb[1][:, c * DCHUNK : (c + 1) * DCHUNK].rearrange(
                    "p (r x) -> p r x", x=HW
                )
                nc.vector.tensor_scalar_add(ob, pb, off_sb)
            # Stream this group's output slice out immediately. Image A rides
            # the Scalar HWDGE ring, image B the Sync ring, so the two output
            # streams (and the input stream) drain in parallel.
            lo_col, hi_col = grp[0] * DCHUNK, (grp[-1] + 1) * DCHUNK
            nc.scalar.dma_start(
                out_ap[b0][:, lo_col:hi_col], o_sb[0][:, lo_col:hi_col]
            )
            nc.sync.dma_start(
                out_ap[b0 + 1][:, lo_col:hi_col], o_sb[1][:, lo_col:hi_col]
            )


def _build_nc():
    global _NC_CACHE
    if _NC_CACHE is not None:
        return _NC_CACHE
    nc = bacc.Bacc(
        "TRN2", target_bir_lowering=False, debug=False, num_devices=NCORES
    )
    xp_ap = nc.dram_tensor("xp", [BPC, CIN, NPAD], F16, kind="ExternalInput").ap()
    w2_ap = nc.dram_tensor("w2", [128, TAPS, K], F16, kind="ExternalInput").ap()
    off_ap = nc.dram_tensor("off", [K, 1], F32, kind="ExternalInput").ap()
    out_ap = nc.dram_tensor("out", [BPC, K, NOUT], F16, kind="ExternalOutput").ap()
    with tile.TileContext(nc) as tc:
        with ExitStack() as ctx:
            _conv_kernel(ctx, tc, out_ap, xp_ap, w2_ap, off_ap)
    nc.compile()
    _NC_CACHE = nc
    return nc


def _prep_inputs(x, weight, offset):
    """Host-side layout prep: pad x (57-stride grid), transpose+duplicate
    weights, downcast both to fp16."""
    x = np.ascontiguousarray(np.asarray(x, dtype=np.float32))
    weight = np.asarray(weight, dtype=np.float32)
    offset = np.asarray(offset, dtype=np.float32)

    xph = np.zeros((B, CIN, NPAD), dtype=np.float16)
    grid = xph[:, :, BASE : BASE + (HW + 2) * HP].reshape(B, CIN, HW + 2, HP)
    grid[:, :, 1 : 1 + HW, 0:HW] = x.astype(np.float16)

    wt = (
        np.ascontiguousarray(weight.transpose(1, 2, 3, 0))
        .reshape(CIN, TAPS, K)
        .astype(np.float16)
    )
    w2 = np.ascontiguousarray(np.concatenate([wt, wt], axis=0))  # [128, 9, 128]
    off = np.ascontiguousarray(offset.reshape(K, 1))
    return xph, w2, off


def kernel(x, weight, offset):
    nc = _build_nc()
    xph, w2, off = _prep_inputs(x, weight, offset)
    in_maps = [
        {"xp": xph[i * BPC : (i + 1) * BPC], "w2": w2, "off": off}
        for i in range(NCORES)
    ]
    res = run_bass_kernel_spmd(nc, in_maps, list(range(NCORES))).results
    out = np.concatenate(
        [
            res[i]["out"].astype(np.float32).reshape(BPC, K, HW, HW)
            for i in range(NCORES)
        ],
        axis=0,
    )
    return out


# revision 8
# speedup vs baseline: 1.0223x; 1.0223x over previous
"""Trainium2 Bass kernel for nn_ConvLayer: 3x3 conv (stride 1, pad 1) + per-channel offset.

Problem: x[32,64,56,56] (*) w[128,64,3,3] + offset[128,1,1] -> out[32,128,56,56], fp32.

Strategy (8 NeuronCores, data-parallel over batch, 4 images/core):
  - Conv as 9 shifted matmuls (one per 3x3 tap) accumulated in fp32 PSUM.
  - CIN=64 -> each tap is a contract-64 matmul = half the 128x128 PE array.
    Two images are processed CONCURRENTLY via 64x128 row tiling: image A's
    channels live in SBUF partitions 0-63 (PE tile (0,0)), image B's in
    partitions 64-127 (PE tile (64,0)). Each accumulates into its own PSUM
    bank, reaching full PE-array packing with no data duplication.
  - All device-side tensors are fp16: x and weights are downcast on host
    (error budget: fp16 products accumulated in fp32 PSUM give ~5e-4 rel
    err vs the 2e-2 gate), and the conv output is stored to HBM as fp16 and
    upcast to fp32 on host. This halves HBM traffic (10.5 -> 5.1 MB/core),
    which removed the DMA-bound head/tail stalls of the fp32 version.
  - Host pre-pads x to a 57-stride grid: ONE zero column is shared between
    consecutive rows (right-pad of row r == left-pad of row r+1), plus one
    zero row above and below. Every tap read is then a single contiguous
    shifted window. Weights are pre-transposed to [cin, tap, k] (lhsT
    layout) and duplicated into both partition halves.
  - Output columns are produced on the padded 57-wide grid; the PSUM->SBUF
    eviction (ScalarE for image A, VectorE for image B) compacts to the
    dense 56-wide grid, fuses the per-channel offset add, and downcasts to
    fp16, so the store DMA is small and fully contiguous.
"""

import numpy as np
from contextlib import ExitStack

import concourse.bass as bass
import concourse.tile as tile
from concourse import bacc, mybir
from concourse.bass_utils import run_bass_kernel_spmd

# Problem constants (hardcoded per contract).
B, CIN, HW, K = 32, 64, 56, 128
NCORES = 8
BPC = B // NCORES          # images per core
HP = HW + 1                # padded row stride: 57 (one shared pad col)
BASE = 1                   # element (row, col) lives at BASE + (row+1)*HP + col
NPAD = BASE + (HW + 2) * HP + 4   # 58 padded rows + tap-read slack: 3312
NOUT = HW * HW             # 3136
ROWS_PER_CHUNK = 7
CHUNK = ROWS_PER_CHUNK * HP     # 399 <= 512 (one PSUM bank, fp32)
DCHUNK = ROWS_PER_CHUNK * HW    # 392 dense output cols per chunk
NCHUNKS = HW // ROWS_PER_CHUNK  # 8
TAPS = 9
F16 = mybir.dt.float16
F32 = mybir.dt.float32

_NC_CACHE = None


def _conv_kernel(ctx: ExitStack, tc: "tile.TileContext", out_ap, xp_ap, w2_ap, off_ap):
    nc = tc.nc
    singles = ctx.enter_context(tc.tile_pool(name="singles", bufs=1))
    xpool = ctx.enter_context(tc.tile_pool(name="xpool", bufs=2))
    opool = ctx.enter_context(tc.tile_pool(name="opool", bufs=2))
    psum = ctx.enter_context(tc.tile_pool(name="psum", bufs=8, space="PSUM"))

    # Chunk groups per PSUM allocation: 2 banks per chunk (2 halves). The
    # first and last groups are single chunks: the first so its (small) input
    # slice lands ASAP and the matmul stream starts early, the last so the
    # end-of-kernel eviction+store tail is short.
    groups = [(0,), (1, 2), (3, 4), (5, 6), (7,)]
    # x-load slices (all on the Sync HWDGE ring, in consumption order, one
    # slice per chunk group). Chunk c reads cols < 399*c + 515.
    xbounds = [0, 515, 1314, 2112, 2910, NPAD]

    # Weights as lhsT [c, tap, k], duplicated across both partition halves.
    # One DMA, FIRST in the Sync ring's order: the x slices share that ring
    # and each SDMA engine drains its ring in order, so the weights complete
    # before the x stream — on a separate ring they would crawl at half rate
    # until ~12us (packet round-robin) and stall the first taps.
    w_sb = singles.tile([128, TAPS, K], F16)
    nc.sync.dma_start(w_sb[:], w2_ap[:])
    off_sb = singles.tile([128, 1], F32)
    nc.scalar.dma_start(off_sb[:], off_ap[:])

    # PE warmup: cheap bf16 matmuls on scratch keep TensorE busy through the
    # whole input-DMA head (~4us: body start ~7us to w+x landing ~11us, worse
    # when an SDMA engine wakes late). This both opens the HAM clock gate
    # (1.2 -> 2.4 GHz at ~3.4us of sustained PE activity) and avoids PE-idle
    # gaps that would reset the HAM window, so the real matmul stream starts
    # fully warm. ~426ns each at the cold clock.
    scratch = singles.tile([128, 512], mybir.dt.bfloat16)
    nc.vector.memset(scratch[:], 0.0)
    ps_warm = psum.tile([128, 512], F32, tag="ps", name="ps_warm")
    for _ in range(9):
        nc.tensor.matmul(
            ps_warm[:], lhsT=scratch[0:64, 0:128], rhs=scratch[0:64, :],
            start=True, stop=True,
        )

    for pair in range(BPC // 2):
        b0 = 2 * pair
        # Both images of the pair side by side: [2, CIN, NPAD] -> [128, NPAD],
        # split into column slices so early chunk groups start ASAP.
        x_t = xpool.tile([128, NPAD], F16, tag="x")
        xsrc = xp_ap[b0 : b0 + 2].rearrange("b c n -> (b c) n")
        for s in range(len(xbounds) - 1):
            nc.sync.dma_start(
                x_t[:, xbounds[s] : xbounds[s + 1]],
                xsrc[:, xbounds[s] : xbounds[s + 1]],
            )
        o_sb = [
            opool.tile([128, NOUT], F16, tag="oA", name=f"oA_{pair}"),
            opool.tile([128, NOUT], F16, tag="oB", name=f"oB_{pair}"),
        ]

        for g, grp in enumerate(groups):
            ps = {}
            for half in (0, 1):
                for c in grp:
                    ps[(half, c)] = psum.tile(
                        [128, CHUNK], F32, tag="ps", name=f"ps_{pair}_{half}_{c}"
                    )
            for t in range(TAPS):
                kh, kw = divmod(t, 3)
                o = kh * HP + kw
                st, sp = (t == 0), (t == TAPS - 1)
                for half in (0, 1):
                    lo, hi = 64 * half, 64 * half + 64
                    for c in grp:
                        nc.tensor.matmul(
                            ps[(half, c)][:],
                            lhsT=w_sb[lo:hi, t, :],
                            rhs=x_t[lo:hi, o + CHUNK * c : o + CHUNK * c + CHUNK],
                            start=st,
                            stop=sp,
                        )
            # Evict: compact 57-stride padded rows to 56-wide dense rows, add
            # the per-channel offset, downcast to fp16. Image A on ScalarE,
            # image B on VectorE (they hit different PSUM banks in parallel).
            # Each chunk's store is dispatched right after its eviction so the
            # output stream never builds an end-of-kernel backlog: the final
            # chunk's store is the only transfer left after the last matmul.
            # Image A rides the Scalar HWDGE ring, image B the Sync ring.
            for c in grp:
                lo_col, hi_col = c * DCHUNK, (c + 1) * DCHUNK
                pa = ps[(0, c)].rearrange("p (r x) -> p r x", x=HP)[:, :, 0:HW]
                oa = o_sb[0][:, lo_col:hi_col].rearrange(
                    "p (r x) -> p r x", x=HW
                )
                nc.scalar.add(oa, pa, off_sb)
                nc.scalar.dma_start(
                    out_ap[b0][:, lo_col:hi_col], o_sb[0][:, lo_col:hi_col]
                )
                pb = ps[(1, c)].rearrange("p (r x) -> p r x", x=HP)[:, :, 0:HW]
                ob = o_sb[1][:, lo_col:hi_col].rearrange(
                    "p (r x) -> p r x", x=HW
                )
                nc.vector.tensor_scalar_add(ob, pb, off_sb)
                nc.sync.dma_start(
                    out_ap[b0 + 1][:, lo_col:hi_col], o_sb[1][:, lo_col:hi_col]
                )


def _build_nc():
    global _NC_CACHE
    if _NC_CACHE is not None:
        return _NC_CACHE
    nc = bacc.Bacc(
        "TRN2", target_bir_lowering=False, debug=False, num_devices=NCORES
    )
    xp_ap = nc.dram_tensor("xp", [BPC, CIN, NPAD], F16, kind="ExternalInput").ap()
    w2_ap = nc.dram_tensor("w2", [128, TAPS, K], F16, kind="ExternalInput").ap()
    off_ap = nc.dram_tensor("off", [K, 1], F32, kind="ExternalInput").ap()
    out_ap = nc.dram_tensor("out", [BPC, K, NOUT], F16, kind="ExternalOutput").ap()
    with tile.TileContext(nc) as tc:
        with ExitStack() as ctx:
            _conv_kernel(ctx, tc, out_ap, xp_ap, w2_ap, off_ap)
    nc.compile()
    _NC_CACHE = nc
    return nc


def _prep_inputs(x, weight, offset):
    """Host-side layout prep: pad x (57-stride grid), transpose+duplicate
    weights, downcast both to fp16."""
    x = np.ascontiguousarray(np.asarray(x, dtype=np.float32))
    weight = np.asarray(weight, dtype=np.float32)
    offset = np.asarray(offset, dtype=np.float32)

    xph = np.zeros((B, CIN, NPAD), dtype=np.float16)
    grid = xph[:, :, BASE : BASE + (HW + 2) * HP].reshape(B, CIN, HW + 2, HP)
    grid[:, :, 1 : 1 + HW, 0:HW] = x.astype(np.float16)

    wt = (
        np.ascontiguousarray(weight.transpose(1, 2, 3, 0))
        .reshape(CIN, TAPS, K)
        .astype(np.float16)
    )
    w2 = np.ascontiguousarray(np.concatenate([wt, wt], axis=0))  # [128, 9, 128]
    off = np.ascontiguousarray(offset.reshape(K, 1))
    return xph, w2, off


def kernel(x, weight, offset):
    nc = _build_nc()
    xph, w2, off = _prep_inputs(x, weight, offset)
    in_maps = [
        {"xp": xph[i * BPC : (i + 1) * BPC], "w2": w2, "off": off}
        for i in range(NCORES)
    ]
    res = run_bass_kernel_spmd(nc, in_maps, list(range(NCORES))).results
    out = np.concatenate(
        [
            res[i]["out"].astype(np.float32).reshape(BPC, K, HW, HW)
            for i in range(NCORES)
        ],
        axis=0,
    )
    return out
